# revision 18
# baseline (speedup 1.0000x reference)
"""HDLoss (haze-density weighted L1) Trainium2 kernel.

Full inputs a, p, n: [16, 3, 512, 512] f32. Output: scalar f32 (mean L1 of
mask*a vs mask*p, where mask is a per-64x64-block coefficient map computed
from |n - a|).

Strategy (pure data parallel, 8 cores, 2 batch images each):
  Device (raw Bass, no Tile): for each of the 6 (b, c) planes per core, one
  3 MB HWDGE DMA brings the a/n/p planes in together; DVE computes
  d = a - n (resp. a - p) and 64-wide segment sums of |d| per partition row
  (tensor_reduce with apply_absolute_value). The per-partition row sums
  R [128, 384] go back to HBM.
  Host: 64-row group sums (the H direction of each 64x64 block) plus the
  remaining mask math run on 8*[128, 384] floats in float64.

Raw Bass is used deliberately: this walrus build rejects instructions with
more than one semaphore wait, which rules out Tile's multi-lane DMA
round-robin and its drain/barrier teardown. The explicit schedule below has
exactly one semaphore wait per instruction and no teardown barrier.

The per-pixel loss term is mask * |a - p| and mask is constant over each
64x64 block, so the exact loss is sum(mb * S) / numel with
  mb = per-block mask coefficients (from |n - a| block sums)
  S  = per-block sums of |a - p|.
"""

import numpy as np

_B, _C, _H, _W = 16, 3, 512, 512
_NCORES = 8
_BLOC = _B // _NCORES            # 2 images per core
_NIMG = _BLOC * _C               # 6 (b, c) planes per core
_SEG = 64                        # block edge
_NT = _H // 128                  # 4 h-tiles of 128 rows per plane
_FREE = _NT * _W                 # 2048 elements per partition per plane
_NBLK = _H // _SEG               # 8 blocks per side
_RCOLS = _NIMG * _NT * _NBLK     # 192 reduce columns per stream
_OUTC = 2 * _RCOLS               # 384


def _build_nc():
    import concourse.bass as bass
    import concourse.mybir as mybir
    from contextlib import ExitStack

    fp32 = mybir.dt.float32
    # Raw-bass program order on a single engine (DVE drains its pipe after
    # every op) provides the same-engine RAW ordering; the race detector has
    # no scheduling metadata to credit it, so it is disabled for this build.
    nc = bass.Bass(detect_race_conditions=False)
    # x = stack([a, n, p]) along axis 2, per-core shard, so the (s, t) DMA
    # dims merge (s stride == 4 * t stride) and the AP stays within 3 dims.
    x_d = nc.dram_tensor("x", [_BLOC, _C, 3, _H, _W], fp32, kind="ExternalInput")
    r_d = nc.dram_tensor("r", [128, _OUTC], fp32, kind="ExternalOutput")

    ctx = ExitStack()
    with ctx:
        txs = [ctx.enter_context(nc.sbuf_tensor(f"tx{k}", [128, 3, _NT, _W], fp32))
               for k in range(_NIMG)]
        d = ctx.enter_context(nc.sbuf_tensor("d", [128, 2 * _FREE], fp32))
        R = ctx.enter_context(nc.sbuf_tensor("R", [128, _OUTC], fp32))
        dsem = ctx.enter_context(nc.semaphore("dsem"))
        esem = ctx.enter_context(nc.semaphore("esem"))
        vsem = ctx.enter_context(nc.semaphore("vsem"))
        block = ctx.enter_context(nc.Block())

        # Loads alternate between the two physical HWDGE rings (SP and ACT)
        # so the per-DMA fixed costs of consecutive transfers overlap. Each
        # ring is FIFO, so per-ring cumulative sem waits stay exact.
        def _load(eng, img, sem):
            b, c = divmod(img, _C)
            eng.dma_start(
                out=txs[img][:],
                in_=x_d[b, c].rearrange("s (t p) w -> p s t w", p=128),
            ).then_inc(sem, 16)

        @block.sync
        def _(sync):
            for img in range(0, _NIMG, 2):
                _load(sync, img, dsem)
            # R complete -> store it, then require the store's completion so
            # the program cannot retire with the DMA in flight.
            sync.wait_ge(vsem, _NIMG)
            sync.dma_start(out=r_d[:], in_=R[:]).then_inc(dsem, 16)
            sync.wait_ge(dsem, 16 * (_NIMG // 2 + 1))

        @block.scalar
        def _(scalar):
            for img in range(1, _NIMG, 2):
                _load(scalar, img, esem)
            scalar.wait_ge(esem, 16 * (_NIMG // 2))

        @block.vector
        def _(vector):
            for img in range(_NIMG):
                sem = dsem if img % 2 == 0 else esem
                vector.wait_ge(sem, 16 * (img // 2 + 1))
                tx = txs[img]
                # One subtract for both streams: broadcast the a-plane
                # against the adjacent n/p planes -> d = [a-n | a-p].
                ta2 = (tx[:, 0].rearrange("p t w -> p (t w)")
                       .rearrange("p (x f) -> p x f", x=1)
                       .broadcast_to((128, 2, _FREE)))
                np2 = tx[:, 1:3].rearrange("p s t w -> p s (t w)")
                vector.tensor_sub(d[:].rearrange("p (s f) -> p s f", f=_FREE),
                                  ta2, np2)
                # One 64-wide segmented abs-reduce for both streams:
                # 64 segments -> R cols [img*64, (img+1)*64).
                red = vector.tensor_reduce(
                    out=R[:, img * 64:(img + 1) * 64],
                    in_=d[:].rearrange("p (s e) -> p s e", e=_SEG),
                    axis=mybir.AxisListType.X,
                    op=mybir.AluOpType.add,
                    apply_absolute_value=True,
                ).then_inc(vsem, 1)

    return nc


_NC_CACHE = None


def _get_nc():
    global _NC_CACHE
    if _NC_CACHE is None:
        _NC_CACHE = _build_nc()
    return _NC_CACHE


def _unpack_core(r):
    """[128, 384] device result -> (blk_an, blk_ap), each [BLOC, C, 8, 8] f64."""
    r = np.asarray(r, dtype=np.float64)
    # partition p = h within 128-row tile; halves of 64 are the block rows.
    o = r.reshape(2, 64, _OUTC).sum(axis=1)          # [m, col]
    # col = img*64 + s*32 + t*8 + j
    v = o.reshape(2, _NIMG, 2, _NT, _NBLK)           # [m, img, s, t, j]
    blks = []
    for s in range(2):
        # [m, img, t, j] -> [img, t, m, j]; block row i = 2*t + m.
        blks.append(v[:, :, s].transpose(1, 2, 0, 3)
                    .reshape(_BLOC, _C, _NBLK, _NBLK))
    return blks[0], blks[1]


def _finish(outs):
    """outs: list of 8 [128, 384] arrays -> scalar f32 loss."""
    blk_list, s_list = [], []
    for o in outs:
        b1, b2 = _unpack_core(o)
        blk_list.append(b1)
        s_list.append(b2)
    blk = np.concatenate(blk_list, axis=0)   # [16, 3, 8, 8] sums of |n - a|
    S = np.concatenate(s_list, axis=0)       # [16, 3, 8, 8] sums of |a - p|

    diff = blk.sum(axis=(2, 3))              # [16, 3]
    ws = (blk[:, :, :-1, :-1] + blk[:, :, 1:, :-1]
          + blk[:, :, :-1, 1:] + blk[:, :, 1:, 1:])  # [16, 3, 7, 7]
    wv = ws / diff[:, :, None, None]

    def pad4(x, di, dj):
        return np.pad(x, ((0, 0), (0, 0), (di, 1 - di), (dj, 1 - dj)))

    mask_blk = pad4(wv, 0, 0) + pad4(wv, 1, 0) + pad4(wv, 0, 1) + pad4(wv, 1, 1)

    ones = np.ones((_NBLK - 1, _NBLK - 1))
    def pad2(x, di, dj):
        return np.pad(x, ((di, 1 - di), (dj, 1 - dj)))
    coeff = pad2(ones, 0, 0) + pad2(ones, 1, 0) + pad2(ones, 0, 1) + pad2(ones, 1, 1)

    mb = mask_blk / coeff                    # [16, 3, 8, 8]
    loss = (mb * S).sum() / float(_B * _C * _H * _W)
    return np.array(loss, dtype=np.float32)


def _shard_inputs(a, p, n):
    in_maps = []
    for i in range(_NCORES):
        sl = slice(_BLOC * i, _BLOC * (i + 1))
        x = np.stack([np.asarray(a[sl], dtype=np.float32),
                      np.asarray(n[sl], dtype=np.float32),
                      np.asarray(p[sl], dtype=np.float32)], axis=2)
        in_maps.append({"x": np.ascontiguousarray(x)})
    return in_maps


def _run(a, p, n, trace=False, **kw):
    """Run the device part; returns (BassKernelResults, [r arrays])."""
    from concourse.bass_utils import run_bass_kernel_spmd
    nc = _get_nc()
    res = run_bass_kernel_spmd(nc, _shard_inputs(a, p, n),
                               list(range(_NCORES)), trace=trace, **kw)
    outs = [res.results[i]["r"] for i in range(_NCORES)]
    return res, outs


def kernel(a, p, n):
    _, outs = _run(a, p, n)
    return _finish(outs)


# revision 19
# speedup vs baseline: 1.1140x; 1.1140x over previous
"""HDLoss (haze-density weighted L1) Trainium2 kernel.

Full inputs a, p, n: [16, 3, 512, 512] f32. Output: scalar f32 (mean L1 of
mask*a vs mask*p, where mask is a per-64x64-block coefficient map computed
from |n - a|).

Strategy (pure data parallel, 8 cores, 2 batch images each):
  Device (raw Bass, no Tile): for each of the 6 (b, c) planes per core, one
  3 MB HWDGE DMA brings the a/n/p planes in together; DVE computes
  d = a - n (resp. a - p) and 64-wide segment sums of |d| per partition row
  (tensor_reduce with apply_absolute_value). The per-partition row sums
  R [128, 384] go back to HBM.
  Host: 64-row group sums (the H direction of each 64x64 block) plus the
  remaining mask math run on 8*[128, 384] floats in float64.

Raw Bass is used deliberately: this walrus build rejects instructions with
more than one semaphore wait, which rules out Tile's multi-lane DMA
round-robin and its drain/barrier teardown. The explicit schedule below has
exactly one semaphore wait per instruction and no teardown barrier.

The per-pixel loss term is mask * |a - p| and mask is constant over each
64x64 block, so the exact loss is sum(mb * S) / numel with
  mb = per-block mask coefficients (from |n - a| block sums)
  S  = per-block sums of |a - p|.
"""

import numpy as np

_B, _C, _H, _W = 16, 3, 512, 512
_NCORES = 8
_BLOC = _B // _NCORES            # 2 images per core
_NIMG = _BLOC * _C               # 6 (b, c) planes per core
_SEG = 64                        # block edge
_NT = _H // 128                  # 4 h-tiles of 128 rows per plane
_FREE = _NT * _W                 # 2048 elements per partition per plane
_NBLK = _H // _SEG               # 8 blocks per side
_RCOLS = _NIMG * _NT * _NBLK     # 192 reduce columns per stream
_OUTC = 2 * _RCOLS               # 384


def _build_nc():
    import concourse.bass as bass
    import concourse.mybir as mybir
    from contextlib import ExitStack

    fp32 = mybir.dt.float32
    # Raw-bass program order on a single engine (DVE drains its pipe after
    # every op) provides the same-engine RAW ordering; the race detector has
    # no scheduling metadata to credit it, so it is disabled for this build.
    nc = bass.Bass(detect_race_conditions=False)
    # x = stack([a, n, p]) along axis 2, per-core shard, so the (s, t) DMA
    # dims merge (s stride == 4 * t stride) and the AP stays within 3 dims.
    x_d = nc.dram_tensor("x", [_BLOC, _C, 3, _H, _W], fp32, kind="ExternalInput")
    r_d = nc.dram_tensor("r", [128, _OUTC], fp32, kind="ExternalOutput")

    ctx = ExitStack()
    with ctx:
        txs = [ctx.enter_context(nc.sbuf_tensor(f"tx{k}", [128, 3, _NT, _W], fp32))
               for k in range(_NIMG)]
        d = ctx.enter_context(nc.sbuf_tensor("d", [128, 2 * _FREE], fp32))
        R = ctx.enter_context(nc.sbuf_tensor("R", [128, _OUTC], fp32))
        dsem = ctx.enter_context(nc.semaphore("dsem"))
        esem = ctx.enter_context(nc.semaphore("esem"))
        vsem = ctx.enter_context(nc.semaphore("vsem"))
        block = ctx.enter_context(nc.Block())

        # Loads alternate between the two physical HWDGE rings (SP and ACT)
        # so the per-DMA fixed costs of consecutive transfers overlap. Each
        # ring is FIFO, so per-ring cumulative sem waits stay exact.
        def _load(eng, img, sem):
            b, c = divmod(img, _C)
            eng.dma_start(
                out=txs[img][:],
                in_=x_d[b, c].rearrange("s (t p) w -> p s t w", p=128),
            ).then_inc(sem, 16)

        # Ring A (sync): img0 as _NT quarter-chunks (DVE starts after ~1/4 of
        # the first transfer), then img3, img4. Ring B (scalar): img1, img2,
        # img5. This assignment leaves no data stalls on the DVE chain.
        @block.sync
        def _(sync):
            src0 = x_d[0, 0].rearrange("s (t p) w -> p s t w", p=128)
            for t in range(_NT):
                sync.dma_start(
                    out=txs[0][:, :, t, :], in_=src0[:, :, t, :]
                ).then_inc(dsem, 16)
            for img in (3, 4):
                _load(sync, img, dsem)
            # R complete -> store it, then require the store's completion so
            # the program cannot retire with the DMA in flight.
            sync.wait_ge(vsem, _NIMG)
            sync.dma_start(out=r_d[:], in_=R[:]).then_inc(dsem, 16)
            sync.wait_ge(dsem, 16 * (_NT + 3))

        @block.scalar
        def _(scalar):
            for img in (1, 2, 5):
                _load(scalar, img, esem)
            scalar.wait_ge(esem, 16 * 3)

        # DVE wait (sem, value) per image; img0 handled per-chunk below.
        _dwait = {1: (esem, 16), 2: (esem, 32), 3: (dsem, 16 * (_NT + 1)),
                  4: (dsem, 16 * (_NT + 2)), 5: (esem, 48)}

        @block.vector
        def _(vector):
            # Image 0: quarter-chunk subtract+reduce as chunks arrive. The
            # reduce output is a strided [128, 2, _NBLK] view of img0's R
            # columns (s-major layout within the image's 64-column group).
            tx = txs[0]
            R0 = R[:, 0:64].rearrange("p (s tj) -> p s tj", s=2)
            dq = d[:, 0:2 * _W].rearrange("p (s f) -> p s f", s=2)
            for t in range(_NT):
                vector.wait_ge(dsem, 16 * (t + 1))
                ta2 = (tx[:, 0, t]
                       .rearrange("p (x f) -> p x f", x=1)
                       .broadcast_to((128, 2, _W)))
                vector.tensor_sub(dq, ta2, tx[:, 1:3, t])
                red = vector.tensor_reduce(
                    out=R0[:, :, t * _NBLK:(t + 1) * _NBLK],
                    in_=dq.rearrange("p s (j e) -> p s j e", e=_SEG),
                    axis=mybir.AxisListType.X,
                    op=mybir.AluOpType.add,
                    apply_absolute_value=True,
                )
                if t == _NT - 1:
                    red.then_inc(vsem, 1)
            for img in range(1, _NIMG):
                sem, val = _dwait[img]
                vector.wait_ge(sem, val)
                tx = txs[img]
                # One subtract for both streams: broadcast the a-plane
                # against the adjacent n/p planes -> d = [a-n | a-p].
                ta2 = (tx[:, 0].rearrange("p t w -> p (t w)")
                       .rearrange("p (x f) -> p x f", x=1)
                       .broadcast_to((128, 2, _FREE)))
                np2 = tx[:, 1:3].rearrange("p s t w -> p s (t w)")
                vector.tensor_sub(d[:].rearrange("p (s f) -> p s f", f=_FREE),
                                  ta2, np2)
                # One 64-wide segmented abs-reduce for both streams:
                # 64 segments -> R cols [img*64, (img+1)*64).
                red = vector.tensor_reduce(
                    out=R[:, img * 64:(img + 1) * 64],
                    in_=d[:].rearrange("p (s e) -> p s e", e=_SEG),
                    axis=mybir.AxisListType.X,
                    op=mybir.AluOpType.add,
                    apply_absolute_value=True,
                ).then_inc(vsem, 1)

    return nc


_NC_CACHE = None


def _get_nc():
    global _NC_CACHE
    if _NC_CACHE is None:
        _NC_CACHE = _build_nc()
    return _NC_CACHE


def _unpack_core(r):
    """[128, 384] device result -> (blk_an, blk_ap), each [BLOC, C, 8, 8] f64."""
    r = np.asarray(r, dtype=np.float64)
    # partition p = h within 128-row tile; halves of 64 are the block rows.
    o = r.reshape(2, 64, _OUTC).sum(axis=1)          # [m, col]
    # col = img*64 + s*32 + t*8 + j
    v = o.reshape(2, _NIMG, 2, _NT, _NBLK)           # [m, img, s, t, j]
    blks = []
    for s in range(2):
        # [m, img, t, j] -> [img, t, m, j]; block row i = 2*t + m.
        blks.append(v[:, :, s].transpose(1, 2, 0, 3)
                    .reshape(_BLOC, _C, _NBLK, _NBLK))
    return blks[0], blks[1]


def _finish(outs):
    """outs: list of 8 [128, 384] arrays -> scalar f32 loss."""
    blk_list, s_list = [], []
    for o in outs:
        b1, b2 = _unpack_core(o)
        blk_list.append(b1)
        s_list.append(b2)
    blk = np.concatenate(blk_list, axis=0)   # [16, 3, 8, 8] sums of |n - a|
    S = np.concatenate(s_list, axis=0)       # [16, 3, 8, 8] sums of |a - p|

    diff = blk.sum(axis=(2, 3))              # [16, 3]
    ws = (blk[:, :, :-1, :-1] + blk[:, :, 1:, :-1]
          + blk[:, :, :-1, 1:] + blk[:, :, 1:, 1:])  # [16, 3, 7, 7]
    wv = ws / diff[:, :, None, None]

    def pad4(x, di, dj):
        return np.pad(x, ((0, 0), (0, 0), (di, 1 - di), (dj, 1 - dj)))

    mask_blk = pad4(wv, 0, 0) + pad4(wv, 1, 0) + pad4(wv, 0, 1) + pad4(wv, 1, 1)

    ones = np.ones((_NBLK - 1, _NBLK - 1))
    def pad2(x, di, dj):
        return np.pad(x, ((di, 1 - di), (dj, 1 - dj)))
    coeff = pad2(ones, 0, 0) + pad2(ones, 1, 0) + pad2(ones, 0, 1) + pad2(ones, 1, 1)

    mb = mask_blk / coeff                    # [16, 3, 8, 8]
    loss = (mb * S).sum() / float(_B * _C * _H * _W)
    return np.array(loss, dtype=np.float32)


def _shard_inputs(a, p, n):
    in_maps = []
    for i in range(_NCORES):
        sl = slice(_BLOC * i, _BLOC * (i + 1))
        x = np.stack([np.asarray(a[sl], dtype=np.float32),
                      np.asarray(n[sl], dtype=np.float32),
                      np.asarray(p[sl], dtype=np.float32)], axis=2)
        in_maps.append({"x": np.ascontiguousarray(x)})
    return in_maps


def _run(a, p, n, trace=False, **kw):
    """Run the device part; returns (BassKernelResults, [r arrays])."""
    from concourse.bass_utils import run_bass_kernel_spmd
    nc = _get_nc()
    res = run_bass_kernel_spmd(nc, _shard_inputs(a, p, n),
                               list(range(_NCORES)), trace=trace, **kw)
    outs = [res.results[i]["r"] for i in range(_NCORES)]
    return res, outs


def kernel(a, p, n):
    _, outs = _run(a, p, n)
    return _finish(outs)
